# revision 23
# baseline (speedup 1.0000x reference)
"""CornerPooling Trainium2 Bass kernel.

Pipeline per image ([512, 512], single channel):
  x1 = relu(a1*conv3x3(x, w3r) + c1)          (conv+BN+relu folded)
  i1 = reverse-cummax over W of x1
  i2 = reverse-cummax over H of x1
  s  = relu(a2*conv3x3(i1+i2, w3b) + a5*x + c25)
  o1 = relu(a1*conv3x3(s, w3r) + c1)
  o2 = relu(conv3x3(o1, w3rr) + b3rr)
  out = w1*o2 + b1

Implementation: data-parallel over 8 NeuronCores (8 images each).
Convs = banded [128,128] fp32r matmuls on the TensorEngine (3 col-shifted
tridiagonal matmuls per 128-row block + single-entry seam matmuls between
blocks). Cummaxes = DVE tensor_tensor_scan with reversed (negative-stride)
APs; the H-direction scan goes through PE transposes (PSUM) and back.
BN/ReLU/bias folding happens in the ACT-engine PSUM evacuation.

I/O is fp16 over the wire (halves the axon transfer cost); compute stays
fp32/fp32r internally.  The jit-compiled SPMD executable is cached at
module level so repeat kernel() calls skip recompilation.
"""

import os
import sys
import time
import numpy as np

for _p in ("/opt/trn_rl_repo",):
    if _p not in sys.path and os.path.isdir(_p):
        sys.path.insert(0, _p)

EPS = 1e-5
N_CORES = 8
IMG_PER_CORE = 8
BATCHES = 1                      # per-round RPC overheads beat split gains
IMG_PER_BATCH = IMG_PER_CORE // BATCHES
H = W = 512
NB = 4  # 128-row blocks per image

# matrix slots in the fp16 cmat tensor; fp32 scalar columns live in cscal
_NMAT = 29
_NSCAL = 8

TRACE = False
LAST_EXEC_NS = None
LAST_RESULTS = None
DEBUG_STAGES = bool(os.environ.get("KBENCH"))

_PROG_CACHE = {}
_RUNNER_CACHE = {}


def _dbg(msg, t0=None):
    if DEBUG_STAGES:
        if t0 is not None:
            print(f"[kstage] {msg}: {time.time()-t0:.3f}s", flush=True)
        else:
            print(f"[kstage] {msg}", flush=True)


def _build_program(n_img):
    import concourse.bass as bass
    import concourse.bacc as bacc
    import concourse.mybir as mybir
    import concourse.tile as tile

    f32 = mybir.dt.float32
    f16 = mybir.dt.float16
    RELU = mybir.ActivationFunctionType.Relu
    MAX = mybir.AluOpType.max
    ADD = mybir.AluOpType.add
    MULT = mybir.AluOpType.mult

    nc = bacc.Bacc()
    x_d = nc.dram_tensor("x", [n_img, H, W], f16, kind="ExternalInput")
    cm_d = nc.dram_tensor("cmat", [128, _NMAT * 128], f16, kind="ExternalInput")
    cs_d = nc.dram_tensor("cscal", [128, _NSCAL], f32, kind="ExternalInput")
    o_d = nc.dram_tensor("out", [n_img, H, W], f16, kind="ExternalOutput")

    with tile.TileContext(nc) as tc, __import__("contextlib").ExitStack() as ctx:
        cm_pool = ctx.enter_context(tc.tile_pool(name="cmat", bufs=1))
        cs_pool = ctx.enter_context(tc.tile_pool(name="cscal", bufs=1))
        zero_pool = ctx.enter_context(tc.tile_pool(name="zeros", bufs=1))
        xt_pool = ctx.enter_context(tc.tile_pool(name="xt", bufs=8))
        x1_pool = ctx.enter_context(tc.tile_pool(name="x1", bufs=8))
        i1_pool = ctx.enter_context(tc.tile_pool(name="i1", bufs=8))
        i2_pool = ctx.enter_context(tc.tile_pool(name="i2T", bufs=8))
        ci_pool = ctx.enter_context(tc.tile_pool(name="ci", bufs=8))
        s_pool = ctx.enter_context(tc.tile_pool(name="s", bufs=8))
        o1_pool = ctx.enter_context(tc.tile_pool(name="o1", bufs=8))
        o2_pool = ctx.enter_context(tc.tile_pool(name="o2", bufs=4))
        res_pool = ctx.enter_context(tc.tile_pool(name="res", bufs=4))
        pconv = ctx.enter_context(tc.tile_pool(name="pconv", bufs=4, space="PSUM"))
        ptr = ctx.enter_context(tc.tile_pool(name="ptr", bufs=2, space="PSUM"))
        ptr2 = ctx.enter_context(tc.tile_pool(name="ptr2", bufs=2, space="PSUM"))

        cmat = cm_pool.tile([128, _NMAT * 128], f16)
        nc.sync.dma_start(cmat[:, :], cm_d[:, :])
        cscal = cs_pool.tile([128, _NSCAL], f32)
        nc.sync.dma_start(cscal[:, :], cs_d[:, :])
        zeros = zero_pool.tile([128, 512], f16)
        nc.vector.memset(zeros[:, :], 0.0)

        def mat(i):
            return cmat[:, i * 128:(i + 1) * 128]

        def scal(j):
            return cscal[:, j:j + 1]

        B1 = [mat(i) for i in range(0, 3)]
        B2 = [mat(i) for i in range(3, 6)]
        B4 = [mat(i) for i in range(6, 9)]
        E1u = [mat(i) for i in range(9, 12)]
        E1d = [mat(i) for i in range(12, 15)]
        E2u = [mat(i) for i in range(15, 18)]
        E2d = [mat(i) for i in range(18, 21)]
        E4u = [mat(i) for i in range(21, 24)]
        E4d = [mat(i) for i in range(24, 27)]
        EMATS = {1: (E1u, E1d), 2: (E2u, E2d), 4: (E4u, E4d)}
        IDENT = mat(27)   # fp16 identity for PE transposes
        IA5 = mat(28)     # a5 * identity
        # scalar columns: 0:c1 1:c25 2:b3rr 3:w1 4:b1 5:a5
        BIAS_C1, BIAS_C25, BIAS_B3RR, W1S, B1S, A5S = (scal(j) for j in range(6))

        # column windows for the 3 kernel-column shifts on unpadded tiles:
        # out[:, c] += B[dc] @ in[:, c+dc-1]; SAME zero-padding means the
        # out-of-range input columns simply drop out of the window.
        #   dc=0: in[:, 0:511] -> out[:, 1:512]
        #   dc=1: in[:, 0:512] -> out[:, 0:512]   (emitted first: start=True)
        #   dc=2: in[:, 1:512] -> out[:, 0:511]
        WIN = {0: ((0, 511), (1, 512)), 1: ((0, 512), (0, 512)),
               2: ((1, 512), (0, 511))}

        def conv(in_tiles, B, eslot, extra_rhs=None, extra_lhs=None):
            """3x3 conv over 4 unpadded [128,512] tiles -> 4 PSUM [128,512]."""
            ps = []
            for b in range(NB):
                p = pconv.tile([128, 512], f32)
                mms = []
                for dc in (1, 0, 2):
                    (i0, i1_), (o0, o1_) = WIN[dc]
                    mms.append((B[dc], in_tiles[b][:, i0:i1_], (o0, o1_)))
                Eu, Ed = EMATS[eslot]
                for dc in range(3):
                    (i0, i1_), (o0, o1_) = WIN[dc]
                    if b > 0:
                        mms.append((Eu[dc], in_tiles[b - 1][:, i0:i1_], (o0, o1_)))
                    if b < NB - 1:
                        mms.append((Ed[dc], in_tiles[b + 1][:, i0:i1_], (o0, o1_)))
                if extra_rhs is not None:
                    mms.append((extra_lhs, extra_rhs[b][:, 0:512], (0, 512)))
                for k, (lhs, rhs, (o0, o1_)) in enumerate(mms):
                    nc.tensor.matmul(
                        p[:, o0:o1_], lhsT=lhs, rhs=rhs,
                        start=(k == 0), stop=(k == len(mms) - 1),
                        skip_group_check=True,
                    )
                ps.append(p)
            return ps

        for img in range(n_img):
            # ---- load ----
            xt = []
            for b in range(NB):
                raw = xt_pool.tile([128, 512], f16)
                nc.sync.dma_start(raw[:, :], x_d[img, 128 * b:128 * (b + 1), :])
                xt.append(raw)

            # ---- conv1 (+BN+relu) ----
            ps = conv(xt, B1, 1)
            x1 = []
            for b in range(NB):
                t = x1_pool.tile([128, 512], f16)
                nc.scalar.activation(t[:, :], ps[b][:, :], RELU, bias=BIAS_C1)
                x1.append(t)

            # ---- i1: reverse cummax along W (free dim) ----
            i1 = []
            for b in range(NB):
                t = i1_pool.tile([128, 512], f16)
                rev_in = x1[b][:, ::-1]
                rev_out = t[:, ::-1]
                nc.vector.tensor_tensor_scan(
                    rev_out, rev_in, rev_in, 0.0, op0=MAX, op1=MAX)
                i1.append(t)

            # ---- i2: transpose -> reverse cummax along H -> transpose back ----
            i2T = []
            for wb in range(NB):
                pT = ptr.tile([128, 512], f16, space="PSUM")
                for hb in range(NB):
                    nc.tensor.transpose(
                        pT[:, hb * 128:(hb + 1) * 128],
                        x1[hb][:, wb * 128:(wb + 1) * 128],
                        IDENT)
                t = i2_pool.tile([128, 512], f16)
                nc.vector.tensor_tensor_scan(
                    t[:, ::-1], pT[:, ::-1], zeros[:, :], 0.0, op0=MAX, op1=MAX)
                i2T.append(t)
            ci = []
            for hb in range(NB):
                p2 = ptr2.tile([128, 512], f16, space="PSUM")
                for wb in range(NB):
                    nc.tensor.transpose(
                        p2[:, wb * 128:(wb + 1) * 128],
                        i2T[wb][:, hb * 128:(hb + 1) * 128],
                        IDENT)
                t = ci_pool.tile([128, 512], f16)
                nc.vector.tensor_add(t[:, :], i1[hb][:, :], p2[:, :])
                ci.append(t)

            # ---- conv2 + a5*x, +c2+c5, relu ----
            ps = conv(ci, B2, 2, extra_rhs=xt, extra_lhs=IA5)
            s = []
            for b in range(NB):
                t = s_pool.tile([128, 512], f16)
                nc.scalar.activation(t[:, :], ps[b][:, :], RELU, bias=BIAS_C25)
                s.append(t)

            # ---- conv3 (same folded weights as conv1) ----
            ps = conv(s, B1, 1)
            o1 = []
            for b in range(NB):
                t = o1_pool.tile([128, 512], f16)
                nc.scalar.activation(t[:, :], ps[b][:, :], RELU, bias=BIAS_C1)
                o1.append(t)

            # ---- conv4 + relu, then w1*o2 + b1 (fp16 out) ----
            ps = conv(o1, B4, 4)
            for b in range(NB):
                t = o2_pool.tile([128, 512], f16)
                nc.scalar.activation(t[:, :], ps[b][:, :], RELU, bias=BIAS_B3RR)
                r = res_pool.tile([128, 512], f16)
                nc.vector.tensor_scalar(
                    r[:, :], t[:, :], W1S, B1S, op0=MULT, op1=ADD)
                nc.sync.dma_start(o_d[img, 128 * b:128 * (b + 1), :], r[:, :])

    nc.finalize()
    return nc


def _get_program(n_img):
    if n_img not in _PROG_CACHE:
        t0 = time.time()
        _PROG_CACHE[n_img] = _build_program(n_img)
        _dbg("build program", t0)
    return _PROG_CACHE[n_img]


def _get_runner(n_img):
    """Build (once) a cached jit-compiled SPMD callable for the program."""
    if n_img in _RUNNER_CACHE:
        return _RUNNER_CACHE[n_img]
    t0 = time.time()
    import jax
    import jax.numpy as jnp
    from jax.sharding import NamedSharding
    import concourse.mybir as mybir
    from concourse.bass2jax import (
        _bass_exec_p,
        install_neuronx_cc_hook,
        partition_id_tensor,
        shard_map,
        Mesh,
        PartitionSpec,
    )

    nc = _get_program(n_img)
    install_neuronx_cc_hook()
    assert nc.dbg_addr is None or not nc.dbg_callbacks

    partition_name = (
        nc.partition_id_tensor.name if nc.partition_id_tensor else None
    )
    in_names, out_names, out_avals, zero_templates = [], [], [], []
    for alloc in nc.m.functions[0].allocations:
        if not isinstance(alloc, mybir.MemoryLocationSet):
            continue
        name = alloc.memorylocations[0].name
        if alloc.kind == "ExternalInput":
            if name != partition_name:
                in_names.append(name)
        elif alloc.kind == "ExternalOutput":
            shape = tuple(alloc.tensor_shape)
            dtype = mybir.dt.np(alloc.dtype)
            out_names.append(name)
            out_avals.append(jax.core.ShapedArray(shape, dtype))
            zero_templates.append((shape, dtype))
    n_params = len(in_names)
    n_outs = len(out_avals)
    all_in_names = list(in_names) + list(out_names)
    if partition_name is not None:
        all_in_names.append(partition_name)
    donate = tuple(range(n_params, n_params + n_outs))

    def _body(*args):
        operands = list(args)
        if partition_name is not None:
            operands.append(partition_id_tensor())
        outs = _bass_exec_p.bind(
            *operands,
            out_avals=tuple(out_avals),
            in_names=tuple(all_in_names),
            out_names=tuple(out_names),
            lowering_input_output_aliases=(),
            sim_require_finite=True,
            sim_require_nnan=True,
            nc=nc,
        )
        return tuple(outs)

    devices = jax.devices()[:N_CORES]
    assert len(devices) == N_CORES
    mesh = Mesh(np.asarray(devices), ("core",))
    in_specs = (PartitionSpec("core"),) * (n_params + n_outs)
    out_specs = (PartitionSpec("core"),) * n_outs
    sharded = jax.jit(
        shard_map(
            _body, mesh=mesh, in_specs=in_specs, out_specs=out_specs,
            check_rep=False,
        ),
        donate_argnums=donate,
        keep_unused=True,
    )
    spec = NamedSharding(mesh, PartitionSpec("core"))
    runner = {
        "sharded": sharded,
        "in_names": in_names,
        "out_names": out_names,
        "in_templates": None,  # filled by _start_aot_compile
        "zero_templates": zero_templates,
        "devices": devices,
        "spec": spec,
        "recycle": None,
        "compiled": None,
        "compile_thread": None,
    }
    _RUNNER_CACHE[n_img] = runner
    _dbg("build runner", t0)
    return runner


def _start_aot_compile(runner, in_shapes):
    """Kick off jit lower+compile on a thread; overlaps the input upload."""
    import threading
    import jax

    if runner["compiled"] is not None or runner["compile_thread"] is not None:
        return
    spec = runner["spec"]
    structs = [
        jax.ShapeDtypeStruct(shp, dt, sharding=spec)
        for shp, dt in in_shapes
    ] + [
        jax.ShapeDtypeStruct((N_CORES * shp[0], *shp[1:]), dt, sharding=spec)
        for shp, dt in runner["zero_templates"]
    ]

    def _compile():
        t0 = time.time()
        try:
            runner["compiled"] = runner["sharded"].lower(*structs).compile()
            _dbg("aot compile", t0)
        except Exception as e:
            _dbg(f"aot compile failed ({e!r}); will use jit path")

    th = threading.Thread(target=_compile, daemon=True)
    th.start()
    runner["compile_thread"] = th


def _upload(runner, per_core_ins, ex):
    """per_core_ins: dict name -> list of N_CORES np arrays (per-core shards).

    Returns dict name -> global jax array. Transfers run on the pool."""
    import jax
    from jax import make_array_from_single_device_arrays as _mk

    devices = runner["devices"]
    spec = runner["spec"]
    names = list(per_core_ins)
    jobs = [
        (name, c, per_core_ins[name][c])
        for name in names for c in range(N_CORES)
    ]
    bufs = list(ex.map(lambda j: jax.device_put(j[2], devices[j[1]]), jobs))
    arrs = {}
    k = 0
    for name in names:
        shard = per_core_ins[name][0]
        gshape = (N_CORES * shard.shape[0], *shard.shape[1:])
        arrs[name] = _mk(gshape, spec, bufs[k:k + N_CORES])
        k += N_CORES
    return arrs


def _fetch(runner, out_arrs):
    from concurrent.futures import ThreadPoolExecutor
    outs = {}
    for i, name in enumerate(runner["out_names"]):
        arr = out_arrs[i]
        host = np.empty(arr.shape, arr.dtype)

        def _get(s):
            host[s.index] = np.asarray(s.data)

        with ThreadPoolExecutor(N_CORES) as ex:
            list(ex.map(_get, arr.addressable_shards))
        outs[name] = host
    return outs


def _run_pipelined(n_img, batches):
    """batches: list of dict name -> list of per-core np shards.

    Pipelines upload/execute/fetch across the batches; returns a list of
    dict name -> np.ndarray global output per batch."""
    from concurrent.futures import ThreadPoolExecutor
    import jax

    runner = _get_runner(n_img)
    nb = len(batches)
    if runner["recycle"] is None or len(runner["recycle"]) != nb:
        runner["recycle"] = [None] * nb

    results = [None] * nb
    with ThreadPoolExecutor(16) as ex:
        # phase 1: upload + dispatch every batch (dispatch is async, so
        # batch N+1's upload and execution overlap batch N's execution)
        inflight = []
        for bi in range(nb):
            t0 = time.time()
            arrs = _upload(runner, batches[bi], ex)
            zeros = runner["recycle"][bi]
            if zeros is None:
                z = [
                    [np.zeros(shp, dt)] * N_CORES
                    for shp, dt in runner["zero_templates"]
                ]
                znames = [f"z{i}" for i in range(len(z))]
                zarrs = _upload(runner, dict(zip(znames, z)), ex)
                zeros = tuple(zarrs[n] for n in znames)
            _dbg(f"upload b{bi}", t0)
            th = runner["compile_thread"]
            if th is not None:
                t0 = time.time()
                th.join()
                runner["compile_thread"] = None
                _dbg("join compile", t0)
            fn = runner["compiled"] or runner["sharded"]
            t0 = time.time()
            out_arrs = fn(*[arrs[n] for n in runner["in_names"]], *zeros)
            _dbg(f"dispatch b{bi}", t0)
            inflight.append(out_arrs)
        # phase 2: block + fetch in order
        for bi, out_arrs in enumerate(inflight):
            t0 = time.time()
            out_arrs = jax.block_until_ready(out_arrs)
            _dbg(f"block b{bi}", t0)
            t0 = time.time()
            results[bi] = _fetch(runner, out_arrs)
            _dbg(f"fetch b{bi}", t0)
            runner["recycle"][bi] = out_arrs
    return results


def _tri(K):
    """lhsT[k,m] = K[k-m+1] band for one column shift: [128,128] fp32."""
    B = np.zeros((128, 128), np.float32)
    for dr in (-1, 0, 1):
        v = K[dr + 1]
        idx = np.arange(128)
        msk = (idx + dr >= 0) & (idx + dr < 128)
        B[idx[msk] + dr, idx[msk]] = v
    return B


def _pack_consts(K1, K2, K4, c1, c25, b3rr, w1, b1, a5):
    mats = []
    for K in (K1, K2, K4):
        for dc in range(3):
            mats.append(_tri(K[:, dc]))
    for K in (K1, K2, K4):
        up = []
        dn = []
        for dc in range(3):
            Eu = np.zeros((128, 128), np.float32)
            Eu[127, 0] = K[0, dc]   # row above block: x_{b-1}[127] -> out row 0
            up.append(Eu)
            Ed = np.zeros((128, 128), np.float32)
            Ed[0, 127] = K[2, dc]   # row below block: x_{b+1}[0] -> out row 127
            dn.append(Ed)
        mats.extend(up)
        mats.extend(dn)
    mats.append(np.eye(128, dtype=np.float32))                    # slot 27: IDENT
    mats.append(np.eye(128, dtype=np.float32) * np.float32(a5))   # slot 28: IA5
    cmat = np.zeros((128, _NMAT * 128), np.float16)
    for i, m in enumerate(mats):
        cmat[:, i * 128:(i + 1) * 128] = m.astype(np.float16)
    cscal = np.zeros((128, _NSCAL), np.float32)
    sc = [c1, c25, b3rr, w1, b1, a5, 0.0, 0.0]
    for j, v in enumerate(sc):
        cscal[:, j] = np.float32(v)
    return cmat, cscal


def kernel(**inputs):
    global LAST_EXEC_NS, LAST_RESULTS
    t_all = time.time()

    # warm the jax/axon backend concurrently with program construction
    import threading

    def _warm():
        try:
            import jax
            devs = jax.devices()[:N_CORES]
            # prime the per-device axon connections + nrt comm init so the
            # real upload/execute don't pay for it
            tiny = np.zeros((1,), np.float16)
            bufs = [jax.device_put(tiny, d) for d in devs]
            jax.block_until_ready(bufs)
        except Exception:
            pass

    warm_t = threading.Thread(target=_warm, daemon=True)
    warm_t.start()

    x = np.ascontiguousarray(np.asarray(inputs["x"], np.float32)).reshape(64, H, W)

    def g(n):
        return np.asarray(inputs[n], np.float32)

    w3r, b3r = g("w3r")[0, 0], g("b3r")[0]
    g3r, be3r, m3r, v3r = g("g3r")[0], g("be3r")[0], g("m3r")[0], g("v3r")[0]
    w3b, b3b = g("w3b")[0, 0], g("b3b")[0]
    g3b, be3b, m3b, v3b = g("g3b")[0], g("be3b")[0], g("m3b")[0], g("v3b")[0]
    w1b, b1b = g("w1b")[0, 0, 0, 0], g("b1b")[0]
    g1b, be1b, m1b, v1b = g("g1b")[0], g("be1b")[0], g("m1b")[0], g("v1b")[0]
    w3rr, b3rr = g("w3rr")[0, 0], g("b3rr")[0]
    w1, b1 = g("w1")[0, 0, 0, 0], g("b1")[0]

    a1 = g3r / np.sqrt(v3r + EPS)
    c1 = a1 * (b3r - m3r) + be3r
    K1 = (a1 * w3r).astype(np.float32)
    a2 = g3b / np.sqrt(v3b + EPS)
    c2 = a2 * (b3b - m3b) + be3b
    K2 = (a2 * w3b).astype(np.float32)
    a5 = g1b * w1b / np.sqrt(v1b + EPS)
    c5 = g1b * (b1b - m1b) / np.sqrt(v1b + EPS) + be1b
    K4 = w3rr.astype(np.float32)

    cmat, cscal = _pack_consts(K1, K2, K4, c1, c2 + c5, b3rr, w1, b1, a5)

    _get_program(IMG_PER_BATCH)   # build BIR while the backend warms
    warm_t.join()
    runner = _get_runner(IMG_PER_BATCH)
    _start_aot_compile(runner, [
        ((N_CORES * IMG_PER_BATCH, H, W), np.float16),
        ((N_CORES * 128, _NMAT * 128), np.float16),
        ((N_CORES * 128, _NSCAL), np.float32),
    ])

    t0 = time.time()
    x16 = x.astype(np.float16).reshape(N_CORES, IMG_PER_CORE, H, W)
    _dbg("host prep", t0)

    batches = []
    for bi in range(BATCHES):
        sl = slice(bi * IMG_PER_BATCH, (bi + 1) * IMG_PER_BATCH)
        batches.append({
            "x": [x16[c, sl] for c in range(N_CORES)],
            "cmat": [cmat] * N_CORES,
            "cscal": [cscal] * N_CORES,
        })
    results = _run_pipelined(IMG_PER_BATCH, batches)

    t0 = time.time()
    out = np.empty((N_CORES, IMG_PER_CORE, H, W), np.float32)
    for bi, res in enumerate(results):
        sl = slice(bi * IMG_PER_BATCH, (bi + 1) * IMG_PER_BATCH)
        out[:, sl] = (
            res["out"]
            .reshape(N_CORES, IMG_PER_BATCH, H, W)
            .astype(np.float32)
        )
    out = out.reshape(64, 1, H, W)
    _dbg("host cast", t0)
    LAST_EXEC_NS = None
    LAST_RESULTS = None
    _dbg("kernel total", t_all)
    return out


def reference_numpy(x_img, consts_args):
    """Host-side mirror of the on-device pipeline, for debugging."""
    (K1, K2, K4, c1, c25, b3rr, w1, b1, a5) = consts_args

    def conv3(z, K):
        zp = np.pad(z, 1)
        out = np.zeros_like(z)
        for dr in (-1, 0, 1):
            for dc in (-1, 0, 1):
                out += K[dr + 1, dc + 1] * zp[1 + dr:513 + dr, 1 + dc:513 + dc]
        return out

    x1 = np.maximum(conv3(x_img, K1) + c1, 0)
    i1 = np.maximum.accumulate(x1[:, ::-1], axis=1)[:, ::-1]
    i2 = np.maximum.accumulate(x1[::-1, :], axis=0)[::-1, :]
    s = np.maximum(conv3(i1 + i2, K2) + a5 * x_img + c25, 0)
    o1 = np.maximum(conv3(s, K1) + c1, 0)
    o2 = np.maximum(conv3(o1, K4) + b3rr, 0)
    return w1 * o2 + b1


# revision 24
# speedup vs baseline: 16.1817x; 16.1817x over previous
"""CornerPooling Trainium2 Bass kernel.

Pipeline per image ([512, 512], single channel):
  x1 = relu(a1*conv3x3(x, w3r) + c1)          (conv+BN+relu folded)
  i1 = reverse-cummax over W of x1
  i2 = reverse-cummax over H of x1
  s  = relu(a2*conv3x3(i1+i2, w3b) + a5*x + c25)
  o1 = relu(a1*conv3x3(s, w3r) + c1)
  o2 = relu(conv3x3(o1, w3rr) + b3rr)
  out = w1*o2 + b1

Implementation: data-parallel over 8 NeuronCores (8 images each).
Convs = banded [128,128] fp32r matmuls on the TensorEngine (3 col-shifted
tridiagonal matmuls per 128-row block + single-entry seam matmuls between
blocks). Cummaxes = DVE tensor_tensor_scan with reversed (negative-stride)
APs; the H-direction scan goes through PE transposes (PSUM) and back.
BN/ReLU/bias folding happens in the ACT-engine PSUM evacuation.

I/O is fp16 over the wire (halves the axon transfer cost); compute stays
fp32/fp32r internally.  The jit-compiled SPMD executable is cached at
module level so repeat kernel() calls skip recompilation.
"""

import os
import sys
import time
import numpy as np

for _p in ("/opt/trn_rl_repo",):
    if _p not in sys.path and os.path.isdir(_p):
        sys.path.insert(0, _p)

EPS = 1e-5
N_CORES = 8
IMG_PER_CORE = 8
BATCHES = 1                      # per-round RPC overheads beat split gains
IMG_PER_BATCH = IMG_PER_CORE // BATCHES
H = W = 512
NB = 4  # 128-row blocks per image

# matrix slots in the fp16 cmat tensor; fp32 scalar columns live in cscal
_NMAT = 29
_NSCAL = 8

TRACE = False
LAST_EXEC_NS = None
LAST_RESULTS = None
DEBUG_STAGES = bool(os.environ.get("KBENCH"))

_PROG_CACHE = {}
_RUNNER_CACHE = {}


def _dbg(msg, t0=None):
    if DEBUG_STAGES:
        if t0 is not None:
            print(f"[kstage] {msg}: {time.time()-t0:.3f}s", flush=True)
        else:
            print(f"[kstage] {msg}", flush=True)


def _build_program(n_img):
    import concourse.bass as bass
    import concourse.bacc as bacc
    import concourse.mybir as mybir
    import concourse.tile as tile

    f32 = mybir.dt.float32
    f16 = mybir.dt.float16
    RELU = mybir.ActivationFunctionType.Relu
    MAX = mybir.AluOpType.max
    ADD = mybir.AluOpType.add
    MULT = mybir.AluOpType.mult

    nc = bacc.Bacc()
    x_d = nc.dram_tensor("x", [n_img, H, W], f16, kind="ExternalInput")
    cm_d = nc.dram_tensor("cmat", [128, _NMAT * 128], f16, kind="ExternalInput")
    cs_d = nc.dram_tensor("cscal", [128, _NSCAL], f32, kind="ExternalInput")
    o_d = nc.dram_tensor("out", [n_img, H, W], f16, kind="ExternalOutput")

    with tile.TileContext(nc) as tc, __import__("contextlib").ExitStack() as ctx:
        cm_pool = ctx.enter_context(tc.tile_pool(name="cmat", bufs=1))
        cs_pool = ctx.enter_context(tc.tile_pool(name="cscal", bufs=1))
        zero_pool = ctx.enter_context(tc.tile_pool(name="zeros", bufs=1))
        xt_pool = ctx.enter_context(tc.tile_pool(name="xt", bufs=8))
        x1_pool = ctx.enter_context(tc.tile_pool(name="x1", bufs=8))
        i1_pool = ctx.enter_context(tc.tile_pool(name="i1", bufs=8))
        i2_pool = ctx.enter_context(tc.tile_pool(name="i2T", bufs=8))
        ci_pool = ctx.enter_context(tc.tile_pool(name="ci", bufs=8))
        s_pool = ctx.enter_context(tc.tile_pool(name="s", bufs=8))
        o1_pool = ctx.enter_context(tc.tile_pool(name="o1", bufs=8))
        o2_pool = ctx.enter_context(tc.tile_pool(name="o2", bufs=4))
        res_pool = ctx.enter_context(tc.tile_pool(name="res", bufs=4))
        pconv = ctx.enter_context(tc.tile_pool(name="pconv", bufs=4, space="PSUM"))
        ptr = ctx.enter_context(tc.tile_pool(name="ptr", bufs=2, space="PSUM"))
        ptr2 = ctx.enter_context(tc.tile_pool(name="ptr2", bufs=2, space="PSUM"))

        cmat = cm_pool.tile([128, _NMAT * 128], f16)
        nc.sync.dma_start(cmat[:, :], cm_d[:, :])
        cscal = cs_pool.tile([128, _NSCAL], f32)
        nc.sync.dma_start(cscal[:, :], cs_d[:, :])
        zeros = zero_pool.tile([128, 512], f16)
        nc.vector.memset(zeros[:, :], 0.0)

        def mat(i):
            return cmat[:, i * 128:(i + 1) * 128]

        def scal(j):
            return cscal[:, j:j + 1]

        B1 = [mat(i) for i in range(0, 3)]
        B2 = [mat(i) for i in range(3, 6)]
        B4 = [mat(i) for i in range(6, 9)]
        E1u = [mat(i) for i in range(9, 12)]
        E1d = [mat(i) for i in range(12, 15)]
        E2u = [mat(i) for i in range(15, 18)]
        E2d = [mat(i) for i in range(18, 21)]
        E4u = [mat(i) for i in range(21, 24)]
        E4d = [mat(i) for i in range(24, 27)]
        EMATS = {1: (E1u, E1d), 2: (E2u, E2d), 4: (E4u, E4d)}
        IDENT = mat(27)   # fp16 identity for PE transposes
        IA5 = mat(28)     # a5 * identity
        # scalar columns: 0:c1 1:c25 2:b3rr 3:w1 4:b1 5:a5
        BIAS_C1, BIAS_C25, BIAS_B3RR, W1S, B1S, A5S = (scal(j) for j in range(6))

        # column windows for the 3 kernel-column shifts on unpadded tiles:
        # out[:, c] += B[dc] @ in[:, c+dc-1]; SAME zero-padding means the
        # out-of-range input columns simply drop out of the window.
        #   dc=0: in[:, 0:511] -> out[:, 1:512]
        #   dc=1: in[:, 0:512] -> out[:, 0:512]   (emitted first: start=True)
        #   dc=2: in[:, 1:512] -> out[:, 0:511]
        WIN = {0: ((0, 511), (1, 512)), 1: ((0, 512), (0, 512)),
               2: ((1, 512), (0, 511))}

        def conv(in_tiles, B, eslot, extra_rhs=None, extra_lhs=None):
            """3x3 conv over 4 unpadded [128,512] tiles -> 4 PSUM [128,512]."""
            ps = []
            for b in range(NB):
                p = pconv.tile([128, 512], f32)
                mms = []
                for dc in (1, 0, 2):
                    (i0, i1_), (o0, o1_) = WIN[dc]
                    mms.append((B[dc], in_tiles[b][:, i0:i1_], (o0, o1_)))
                Eu, Ed = EMATS[eslot]
                for dc in range(3):
                    (i0, i1_), (o0, o1_) = WIN[dc]
                    if b > 0:
                        mms.append((Eu[dc], in_tiles[b - 1][:, i0:i1_], (o0, o1_)))
                    if b < NB - 1:
                        mms.append((Ed[dc], in_tiles[b + 1][:, i0:i1_], (o0, o1_)))
                if extra_rhs is not None:
                    mms.append((extra_lhs, extra_rhs[b][:, 0:512], (0, 512)))
                for k, (lhs, rhs, (o0, o1_)) in enumerate(mms):
                    nc.tensor.matmul(
                        p[:, o0:o1_], lhsT=lhs, rhs=rhs,
                        start=(k == 0), stop=(k == len(mms) - 1),
                        skip_group_check=True,
                    )
                ps.append(p)
            return ps

        for img in range(n_img):
            # ---- load ----
            xt = []
            for b in range(NB):
                raw = xt_pool.tile([128, 512], f16)
                nc.sync.dma_start(raw[:, :], x_d[img, 128 * b:128 * (b + 1), :])
                xt.append(raw)

            # ---- conv1 (+BN+relu) ----
            ps = conv(xt, B1, 1)
            x1 = []
            for b in range(NB):
                t = x1_pool.tile([128, 512], f16)
                nc.scalar.activation(t[:, :], ps[b][:, :], RELU, bias=BIAS_C1)
                x1.append(t)

            # ---- i1: reverse cummax along W (free dim) ----
            i1 = []
            for b in range(NB):
                t = i1_pool.tile([128, 512], f16)
                rev_in = x1[b][:, ::-1]
                rev_out = t[:, ::-1]
                nc.vector.tensor_tensor_scan(
                    rev_out, rev_in, rev_in, 0.0, op0=MAX, op1=MAX)
                i1.append(t)

            # ---- i2: transpose -> reverse cummax along H -> transpose back ----
            i2T = []
            for wb in range(NB):
                pT = ptr.tile([128, 512], f16, space="PSUM")
                for hb in range(NB):
                    nc.tensor.transpose(
                        pT[:, hb * 128:(hb + 1) * 128],
                        x1[hb][:, wb * 128:(wb + 1) * 128],
                        IDENT)
                t = i2_pool.tile([128, 512], f16)
                nc.vector.tensor_tensor_scan(
                    t[:, ::-1], pT[:, ::-1], zeros[:, :], 0.0, op0=MAX, op1=MAX)
                i2T.append(t)
            ci = []
            for hb in range(NB):
                p2 = ptr2.tile([128, 512], f16, space="PSUM")
                for wb in range(NB):
                    nc.tensor.transpose(
                        p2[:, wb * 128:(wb + 1) * 128],
                        i2T[wb][:, hb * 128:(hb + 1) * 128],
                        IDENT)
                t = ci_pool.tile([128, 512], f16)
                nc.vector.tensor_add(t[:, :], i1[hb][:, :], p2[:, :])
                ci.append(t)

            # ---- conv2 + a5*x, +c2+c5, relu ----
            ps = conv(ci, B2, 2, extra_rhs=xt, extra_lhs=IA5)
            s = []
            for b in range(NB):
                t = s_pool.tile([128, 512], f16)
                nc.scalar.activation(t[:, :], ps[b][:, :], RELU, bias=BIAS_C25)
                s.append(t)

            # ---- conv3 (same folded weights as conv1) ----
            ps = conv(s, B1, 1)
            o1 = []
            for b in range(NB):
                t = o1_pool.tile([128, 512], f16)
                nc.scalar.activation(t[:, :], ps[b][:, :], RELU, bias=BIAS_C1)
                o1.append(t)

            # ---- conv4 + relu, then w1*o2 + b1 (fp16 out) ----
            ps = conv(o1, B4, 4)
            for b in range(NB):
                t = o2_pool.tile([128, 512], f16)
                nc.scalar.activation(t[:, :], ps[b][:, :], RELU, bias=BIAS_B3RR)
                r = res_pool.tile([128, 512], f16)
                nc.vector.tensor_scalar(
                    r[:, :], t[:, :], W1S, B1S, op0=MULT, op1=ADD)
                nc.sync.dma_start(o_d[img, 128 * b:128 * (b + 1), :], r[:, :])

    nc.finalize()
    return nc


def _get_program(n_img):
    if n_img not in _PROG_CACHE:
        t0 = time.time()
        _PROG_CACHE[n_img] = _build_program(n_img)
        _dbg("build program", t0)
    return _PROG_CACHE[n_img]


def _get_runner(n_img):
    """Build (once) a cached jit-compiled SPMD callable for the program."""
    if n_img in _RUNNER_CACHE:
        return _RUNNER_CACHE[n_img]
    t0 = time.time()
    import jax
    import jax.numpy as jnp
    from jax.sharding import NamedSharding
    import concourse.mybir as mybir
    from concourse.bass2jax import (
        _bass_exec_p,
        install_neuronx_cc_hook,
        partition_id_tensor,
        shard_map,
        Mesh,
        PartitionSpec,
    )

    nc = _get_program(n_img)
    install_neuronx_cc_hook()
    assert nc.dbg_addr is None or not nc.dbg_callbacks

    partition_name = (
        nc.partition_id_tensor.name if nc.partition_id_tensor else None
    )
    in_names, out_names, out_avals, zero_templates = [], [], [], []
    for alloc in nc.m.functions[0].allocations:
        if not isinstance(alloc, mybir.MemoryLocationSet):
            continue
        name = alloc.memorylocations[0].name
        if alloc.kind == "ExternalInput":
            if name != partition_name:
                in_names.append(name)
        elif alloc.kind == "ExternalOutput":
            shape = tuple(alloc.tensor_shape)
            dtype = mybir.dt.np(alloc.dtype)
            out_names.append(name)
            out_avals.append(jax.core.ShapedArray(shape, dtype))
            zero_templates.append((shape, dtype))
    n_params = len(in_names)
    n_outs = len(out_avals)
    all_in_names = list(in_names) + list(out_names)
    if partition_name is not None:
        all_in_names.append(partition_name)
    donate = tuple(range(n_params, n_params + n_outs))

    def _body(*args):
        operands = list(args)
        if partition_name is not None:
            operands.append(partition_id_tensor())
        outs = _bass_exec_p.bind(
            *operands,
            out_avals=tuple(out_avals),
            in_names=tuple(all_in_names),
            out_names=tuple(out_names),
            lowering_input_output_aliases=(),
            sim_require_finite=True,
            sim_require_nnan=True,
            nc=nc,
        )
        return tuple(outs)

    devices = jax.devices()[:N_CORES]
    assert len(devices) == N_CORES
    mesh = Mesh(np.asarray(devices), ("core",))
    in_specs = (PartitionSpec("core"),) * (n_params + n_outs)
    out_specs = (PartitionSpec("core"),) * n_outs
    sharded = jax.jit(
        shard_map(
            _body, mesh=mesh, in_specs=in_specs, out_specs=out_specs,
            check_rep=False,
        ),
        donate_argnums=donate,
        keep_unused=True,
    )
    spec = NamedSharding(mesh, PartitionSpec("core"))
    runner = {
        "sharded": sharded,
        "in_names": in_names,
        "out_names": out_names,
        "in_templates": None,  # filled by _start_aot_compile
        "zero_templates": zero_templates,
        "devices": devices,
        "spec": spec,
        "recycle": None,
        "compiled": None,
        "compile_thread": None,
    }
    _RUNNER_CACHE[n_img] = runner
    _dbg("build runner", t0)
    return runner


def _start_aot_compile(runner, in_shapes):
    """Kick off jit lower+compile on a thread; overlaps the input upload."""
    import threading
    import jax

    if runner["compiled"] is not None or runner["compile_thread"] is not None:
        return
    spec = runner["spec"]
    structs = [
        jax.ShapeDtypeStruct(shp, dt, sharding=spec)
        for shp, dt in in_shapes
    ] + [
        jax.ShapeDtypeStruct((N_CORES * shp[0], *shp[1:]), dt, sharding=spec)
        for shp, dt in runner["zero_templates"]
    ]

    def _compile():
        t0 = time.time()
        try:
            runner["compiled"] = runner["sharded"].lower(*structs).compile()
            _dbg("aot compile", t0)
        except Exception as e:
            _dbg(f"aot compile failed ({e!r}); will use jit path")

    th = threading.Thread(target=_compile, daemon=True)
    th.start()
    runner["compile_thread"] = th


def _upload(runner, per_core_ins, ex):
    """per_core_ins: dict name -> list of N_CORES np arrays (per-core shards).

    Returns dict name -> global jax array. Transfers run on the pool."""
    import jax
    from jax import make_array_from_single_device_arrays as _mk

    devices = runner["devices"]
    spec = runner["spec"]
    names = list(per_core_ins)
    jobs = [
        (name, c, per_core_ins[name][c])
        for name in names for c in range(N_CORES)
    ]
    bufs = list(ex.map(lambda j: jax.device_put(j[2], devices[j[1]]), jobs))
    arrs = {}
    k = 0
    for name in names:
        shard = per_core_ins[name][0]
        gshape = (N_CORES * shard.shape[0], *shard.shape[1:])
        arrs[name] = _mk(gshape, spec, bufs[k:k + N_CORES])
        k += N_CORES
    return arrs


def _fetch(runner, out_arrs):
    from concurrent.futures import ThreadPoolExecutor
    outs = {}
    for i, name in enumerate(runner["out_names"]):
        arr = out_arrs[i]
        host = np.empty(arr.shape, arr.dtype)

        def _get(s):
            host[s.index] = np.asarray(s.data)

        with ThreadPoolExecutor(N_CORES) as ex:
            list(ex.map(_get, arr.addressable_shards))
        outs[name] = host
    return outs


def _run_pipelined(n_img, batches):
    """batches: list of dict name -> list of per-core np shards.

    Pipelines upload/execute/fetch across the batches; returns a list of
    dict name -> np.ndarray global output per batch."""
    from concurrent.futures import ThreadPoolExecutor
    import jax

    runner = _get_runner(n_img)
    nb = len(batches)
    if runner["recycle"] is None or len(runner["recycle"]) != nb:
        runner["recycle"] = [None] * nb

    results = [None] * nb
    with ThreadPoolExecutor(16) as ex:
        # phase 1: upload + dispatch every batch (dispatch is async, so
        # batch N+1's upload and execution overlap batch N's execution)
        inflight = []
        for bi in range(nb):
            t0 = time.time()
            arrs = _upload(runner, batches[bi], ex)
            zeros = runner["recycle"][bi]
            if zeros is None:
                z = [
                    [np.zeros(shp, dt)] * N_CORES
                    for shp, dt in runner["zero_templates"]
                ]
                znames = [f"z{i}" for i in range(len(z))]
                zarrs = _upload(runner, dict(zip(znames, z)), ex)
                zeros = tuple(zarrs[n] for n in znames)
            _dbg(f"upload b{bi}", t0)
            th = runner["compile_thread"]
            if th is not None:
                t0 = time.time()
                th.join()
                runner["compile_thread"] = None
                _dbg("join compile", t0)
            fn = runner["compiled"] or runner["sharded"]
            t0 = time.time()
            out_arrs = fn(*[arrs[n] for n in runner["in_names"]], *zeros)
            _dbg(f"dispatch b{bi}", t0)
            inflight.append(out_arrs)
        # phase 2: block + fetch in order
        for bi, out_arrs in enumerate(inflight):
            t0 = time.time()
            out_arrs = jax.block_until_ready(out_arrs)
            _dbg(f"block b{bi}", t0)
            t0 = time.time()
            results[bi] = _fetch(runner, out_arrs)
            _dbg(f"fetch b{bi}", t0)
            runner["recycle"][bi] = out_arrs
    return results


def _tri(K):
    """lhsT[k,m] = K[k-m+1] band for one column shift: [128,128] fp32."""
    B = np.zeros((128, 128), np.float32)
    for dr in (-1, 0, 1):
        v = K[dr + 1]
        idx = np.arange(128)
        msk = (idx + dr >= 0) & (idx + dr < 128)
        B[idx[msk] + dr, idx[msk]] = v
    return B


def _pack_consts(K1, K2, K4, c1, c25, b3rr, w1, b1, a5):
    mats = []
    for K in (K1, K2, K4):
        for dc in range(3):
            mats.append(_tri(K[:, dc]))
    for K in (K1, K2, K4):
        up = []
        dn = []
        for dc in range(3):
            Eu = np.zeros((128, 128), np.float32)
            Eu[127, 0] = K[0, dc]   # row above block: x_{b-1}[127] -> out row 0
            up.append(Eu)
            Ed = np.zeros((128, 128), np.float32)
            Ed[0, 127] = K[2, dc]   # row below block: x_{b+1}[0] -> out row 127
            dn.append(Ed)
        mats.extend(up)
        mats.extend(dn)
    mats.append(np.eye(128, dtype=np.float32))                    # slot 27: IDENT
    mats.append(np.eye(128, dtype=np.float32) * np.float32(a5))   # slot 28: IA5
    cmat = np.zeros((128, _NMAT * 128), np.float16)
    for i, m in enumerate(mats):
        cmat[:, i * 128:(i + 1) * 128] = m.astype(np.float16)
    cscal = np.zeros((128, _NSCAL), np.float32)
    sc = [c1, c25, b3rr, w1, b1, a5, 0.0, 0.0]
    for j, v in enumerate(sc):
        cscal[:, j] = np.float32(v)
    return cmat, cscal


def kernel(**inputs):
    global LAST_EXEC_NS, LAST_RESULTS
    t_all = time.time()

    # warm the jax/axon backend concurrently with program construction
    import threading

    def _warm():
        try:
            import jax
            jax.devices()
        except Exception:
            pass

    warm_t = threading.Thread(target=_warm, daemon=True)
    warm_t.start()

    x = np.ascontiguousarray(np.asarray(inputs["x"], np.float32)).reshape(64, H, W)

    def g(n):
        return np.asarray(inputs[n], np.float32)

    w3r, b3r = g("w3r")[0, 0], g("b3r")[0]
    g3r, be3r, m3r, v3r = g("g3r")[0], g("be3r")[0], g("m3r")[0], g("v3r")[0]
    w3b, b3b = g("w3b")[0, 0], g("b3b")[0]
    g3b, be3b, m3b, v3b = g("g3b")[0], g("be3b")[0], g("m3b")[0], g("v3b")[0]
    w1b, b1b = g("w1b")[0, 0, 0, 0], g("b1b")[0]
    g1b, be1b, m1b, v1b = g("g1b")[0], g("be1b")[0], g("m1b")[0], g("v1b")[0]
    w3rr, b3rr = g("w3rr")[0, 0], g("b3rr")[0]
    w1, b1 = g("w1")[0, 0, 0, 0], g("b1")[0]

    a1 = g3r / np.sqrt(v3r + EPS)
    c1 = a1 * (b3r - m3r) + be3r
    K1 = (a1 * w3r).astype(np.float32)
    a2 = g3b / np.sqrt(v3b + EPS)
    c2 = a2 * (b3b - m3b) + be3b
    K2 = (a2 * w3b).astype(np.float32)
    a5 = g1b * w1b / np.sqrt(v1b + EPS)
    c5 = g1b * (b1b - m1b) / np.sqrt(v1b + EPS) + be1b
    K4 = w3rr.astype(np.float32)

    cmat, cscal = _pack_consts(K1, K2, K4, c1, c2 + c5, b3rr, w1, b1, a5)

    _get_program(IMG_PER_BATCH)   # build BIR while the backend warms
    warm_t.join()
    runner = _get_runner(IMG_PER_BATCH)
    _start_aot_compile(runner, [
        ((N_CORES * IMG_PER_BATCH, H, W), np.float16),
        ((N_CORES * 128, _NMAT * 128), np.float16),
        ((N_CORES * 128, _NSCAL), np.float32),
    ])

    t0 = time.time()
    x16 = x.astype(np.float16).reshape(N_CORES, IMG_PER_CORE, H, W)
    _dbg("host prep", t0)

    batches = []
    for bi in range(BATCHES):
        sl = slice(bi * IMG_PER_BATCH, (bi + 1) * IMG_PER_BATCH)
        batches.append({
            "x": [x16[c, sl] for c in range(N_CORES)],
            "cmat": [cmat] * N_CORES,
            "cscal": [cscal] * N_CORES,
        })
    results = _run_pipelined(IMG_PER_BATCH, batches)

    t0 = time.time()
    out = np.empty((N_CORES, IMG_PER_CORE, H, W), np.float32)
    for bi, res in enumerate(results):
        sl = slice(bi * IMG_PER_BATCH, (bi + 1) * IMG_PER_BATCH)
        out[:, sl] = (
            res["out"]
            .reshape(N_CORES, IMG_PER_BATCH, H, W)
            .astype(np.float32)
        )
    out = out.reshape(64, 1, H, W)
    _dbg("host cast", t0)
    LAST_EXEC_NS = None
    LAST_RESULTS = None
    _dbg("kernel total", t_all)
    return out


def reference_numpy(x_img, consts_args):
    """Host-side mirror of the on-device pipeline, for debugging."""
    (K1, K2, K4, c1, c25, b3rr, w1, b1, a5) = consts_args

    def conv3(z, K):
        zp = np.pad(z, 1)
        out = np.zeros_like(z)
        for dr in (-1, 0, 1):
            for dc in (-1, 0, 1):
                out += K[dr + 1, dc + 1] * zp[1 + dr:513 + dr, 1 + dc:513 + dc]
        return out

    x1 = np.maximum(conv3(x_img, K1) + c1, 0)
    i1 = np.maximum.accumulate(x1[:, ::-1], axis=1)[:, ::-1]
    i2 = np.maximum.accumulate(x1[::-1, :], axis=0)[::-1, :]
    s = np.maximum(conv3(i1 + i2, K2) + a5 * x_img + c25, 0)
    o1 = np.maximum(conv3(s, K1) + c1, 0)
    o2 = np.maximum(conv3(o1, K4) + b3rr, 0)
    return w1 * o2 + b1
